# revision 6
# baseline (speedup 1.0000x reference)
"""Multi-head self-attention (B=4, T=2048, D=1024, H=16) on 8 trn2 cores — v2.

Sharding: core = b * 2 + g (b = batch, g = head-group of 8 heads).
All PE operands bf16 (f32 PSUM accumulation). Per core:
  Phase 1: Q^T,K^T [d, T] via stationary-weight matmuls (LDW amortized over
           4 t-chunks); V [t, d] tiles via stationary-x chunks.
  Phase 2: per (head-pair ti, t-block tb): 16 s-iters of
           - scores: 2 row-tiled (K=64) concurrent matmuls -> [128s, 512t] x2
           - exp: ACT exact (exp with scale+bias) or DVE 1-pass i16
             Schraudolph (bitcast bf16), per-si schedule; all at a common
             2^-63/c2 output scale (cancels in softmax).
           - PV: 2 col-tiled concurrent matmuls (M=64 each) accumulating a
             head-pair ctx [128, 512] in one PSUM bank (start-once).
           - denominators: 2 col-tiled M=1 ones-matmuls into a shared bank
             at parity positions (0,32)/(64,96), start-once.
           Normalization: bit-trick reciprocal seed + 2 Newton iterations on
           DVE, gpsimd partition_broadcast, DVE multiply -> ctx bf16.
  Phase 3: out-projection from stationary ctx chunks (2 matmuls per LDW),
           DVE bias add, DMA out. Host sums the two head-group partials.
"""

import numpy as np
import ml_dtypes
import concourse.bass as bass
import concourse.bacc as bacc
import concourse.mybir as mybir
import concourse.tile as tile
from concourse.bass_utils import run_bass_kernel_spmd

B, T, D = 4, 2048, 1024
H, DK = 16, 64
G = 2
HPG = H // G          # 8 heads per core
HD = HPG * DK         # 512
NCORES = B * G
SCALE = 1.0 / float(np.sqrt(DK))

F32 = mybir.dt.float32
BF16 = mybir.dt.bfloat16
I32 = mybir.dt.int32
I16 = mybir.dt.int16
AT = mybir.AluOpType
Ident = mybir.ActivationFunctionType.Identity
Exp = mybir.ActivationFunctionType.Exp

NCC = D // 128        # 8 contraction chunks
NDT = HD // 128       # 4 head-pair tiles
NTT = T // 128        # 16 t-tiles
NSI = T // 128        # 16 s-tiles
NTB = T // 512        # 4 t-blocks

# ---- exp constants (common output scale 2^-63/c2, cancels in softmax) ----
LOG2E = float(np.log2(np.e))
_ws = np.linspace(1, 2, 4001)
_c2, _c1, _c0 = np.polyfit(_ws, 2 ** (_ws - 1) / _ws, 2)
A16 = float((2 ** 7) * LOG2E * SCALE)
B16S = float((127 - 63) * 2 ** 7 + (2 ** 7) * np.log2(1.0 / _c2) - 7.25)
ACT_BIAS = float(-63 * np.log(2) - np.log(_c2))
C_RECIP = 2129850000.0
NR_SCALAR = 2.0 * (1.0 + 0.00066)
# si with both exp halves on ACT (rest split ACT/DVE by half parity)
ACT_BOTH = {1, 4, 7, 9, 12, 15}


def build_program():
    nc = bacc.Bacc("TRN2", target_bir_lowering=False, debug=False)

    xt = nc.dram_tensor("xt", [D, T], BF16, kind="ExternalInput").ap()
    wq = nc.dram_tensor("wq", [D, HD], BF16, kind="ExternalInput").ap()
    wk = nc.dram_tensor("wk", [D, HD], BF16, kind="ExternalInput").ap()
    wv = nc.dram_tensor("wv", [D, HD], BF16, kind="ExternalInput").ap()
    bq = nc.dram_tensor("bq", [HD, 1], F32, kind="ExternalInput").ap()
    bk = nc.dram_tensor("bk", [HD, 1], F32, kind="ExternalInput").ap()
    bv = nc.dram_tensor("bv", [128, HD], F32, kind="ExternalInput").ap()
    wo = nc.dram_tensor("wo", [HD, D], BF16, kind="ExternalInput").ap()
    bo = nc.dram_tensor("bo", [128, D], F32, kind="ExternalInput").ap()
    y = nc.dram_tensor("y", [T, D], BF16, kind="ExternalOutput").ap()

    with tile.TileContext(nc) as tc:
        with tc.tile_pool(name="persist", bufs=1) as pp:
            qT = [pp.tile([128, T], BF16, name=f"qT{i}", tag=f"qT{i}")
                  for i in range(NDT)]
            kT = [pp.tile([128, T], BF16, name=f"kT{i}", tag=f"kT{i}")
                  for i in range(NDT)]
            VW = HPG * (DK + 1)   # 520
            vv = [pp.tile([128, VW], BF16, name=f"v{i}", tag=f"v{i}")
                  for i in range(NSI)]
            ctx = [pp.tile([128, T], BF16, name=f"ctx{i}", tag=f"ctx{i}")
                   for i in range(NDT)]
            ones_sb = pp.tile([128, 1], BF16, name="ones", tag="ones")
            nc.vector.memset(ones_sb[:], 1.0)
            actbias_sb = pp.tile([128, 1], F32, name="actbias", tag="actbias")
            nc.vector.memset(actbias_sb[:], ACT_BIAS)
            onesw_sb = pp.tile([128, HPG], BF16, name="onesw", tag="onesw")
            nc.vector.memset(onesw_sb[:], 1.0)

            # ============ Phase 1: QKV projections ============
            with tc.tile_pool(name="p1", bufs=1) as p1:
                xts = [p1.tile([128, T], BF16, name=f"xt{c}", tag=f"xt{c}")
                       for c in range(NCC)]
                wq_sb = [p1.tile([128, HD], BF16, name=f"wq{c}", tag=f"wq{c}")
                         for c in range(NCC)]
                wk_sb = [p1.tile([128, HD], BF16, name=f"wk{c}", tag=f"wk{c}")
                         for c in range(NCC)]
                wv_sb = [p1.tile([128, HD], BF16, name=f"wv{c}", tag=f"wv{c}")
                         for c in range(NCC)]
                for c in range(NCC):
                    nc.sync.dma_start(wq_sb[c][:], wq[c * 128:(c + 1) * 128, :])
                    eng = nc.sync if c % 2 == 0 else nc.scalar
                    eng.dma_start(xts[c][:], xt[c * 128:(c + 1) * 128, :])
                for c in range(NCC):
                    nc.sync.dma_start(wk_sb[c][:], wk[c * 128:(c + 1) * 128, :])
                    nc.scalar.dma_start(wv_sb[c][:], wv[c * 128:(c + 1) * 128, :])
                bq_sb = [p1.tile([128, 1], F32, name=f"bq{i}", tag=f"bq{i}")
                         for i in range(NDT)]
                bk_sb = [p1.tile([128, 1], F32, name=f"bk{i}", tag=f"bk{i}")
                         for i in range(NDT)]
                for i in range(NDT):
                    nc.sync.dma_start(bq_sb[i][:], bq[i * 128:(i + 1) * 128, :])
                    nc.sync.dma_start(bk_sb[i][:], bk[i * 128:(i + 1) * 128, :])
                bv_sb = p1.tile([128, HD], F32, name="bv_sb", tag="bv_sb")
                nc.sync.dma_start(bv_sb[:], bv[:])

                # Q^T / K^T: stationary w chunk reused across 4 t-chunks
                with tc.tile_pool(name="p1ps", bufs=2, space="PSUM") as p1ps:
                    for w_sb, b_sb, outT in ((wq_sb, bq_sb, qT),
                                             (wk_sb, bk_sb, kT)):
                        for dt in range(NDT):
                            ps = p1ps.tile([128, T], F32, name="qk_ps",
                                           tag="qk_ps")
                            for c in range(NCC):
                                for tcn in range(4):
                                    nc.tensor.matmul(
                                        ps[:, tcn * 512:(tcn + 1) * 512],
                                        w_sb[c][:, dt * 128:(dt + 1) * 128],
                                        xts[c][:, tcn * 512:(tcn + 1) * 512],
                                        start=(c == 0), stop=(c == NCC - 1))
                            nc.scalar.activation(outT[dt][:], ps[:], Ident,
                                                 bias=b_sb[dt][:])
                # V tiles
                with tc.tile_pool(name="p1psv", bufs=3, space="PSUM") as p1psv:
                    for si in range(NSI):
                        psv = p1psv.tile([128, HD], F32, name="v_ps",
                                         tag="v_ps")
                        for c in range(NCC):
                            nc.tensor.matmul(
                                psv[:],
                                xts[c][:, si * 128:(si + 1) * 128],
                                wv_sb[c][:],
                                start=(c == 0), stop=(c == NCC - 1))
                        v3 = vv[si][:].rearrange("p (h e) -> p h e", e=DK + 1)
                        with nc.allow_low_precision(reason="bf16 V tiles"):
                            nc.vector.tensor_add(
                                v3[:, :, 0:DK],
                                psv[:].rearrange("p (h e) -> p h e", e=DK),
                                bv_sb[:].rearrange("p (h e) -> p h e", e=DK))
                        nc.vector.tensor_copy(
                            v3[:, :, DK:DK + 1],
                            onesw_sb[:, 0:HPG].rearrange("p (h e) -> p h e",
                                                         e=1))

            # ---- phase-3 resources (emitted inline during last hp) ----
            wo_sb = [pp.tile([128, D], BF16, name=f"wo{c}", tag=f"wo{c}")
                     for c in range(NDT)]
            for c in range(NDT):
                nc.sync.dma_start(wo_sb[c][:], wo[c * 128:(c + 1) * 128, :])
            bo_sb = pp.tile([128, D], F32, name="bo_sb", tag="bo_sb")
            nc.sync.dma_start(bo_sb[:], bo[:])

            # ============ Phase 2: attention ============
            with tc.tile_pool(name="p2", bufs=1) as p2, \
                 tc.tile_pool(name="p2sp", bufs=4, space="PSUM") as p2sp, \
                 tc.tile_pool(name="p2pc", bufs=4, space="PSUM") as p2pc:

                def emit_proj(tt):
                    py = p2sp.tile([128, 512], F32, name="sp", tag="sp")
                    py2 = p2sp.tile([128, 512], F32, name="sp", tag="sp")
                    for ci in range(NDT):
                        nc.tensor.matmul(
                            py[:],
                            ctx[ci][:, tt * 128:(tt + 1) * 128],
                            wo_sb[ci][:, 0:512],
                            start=(ci == 0), stop=(ci == NDT - 1))
                        nc.tensor.matmul(
                            py2[:],
                            ctx[ci][:, tt * 128:(tt + 1) * 128],
                            wo_sb[ci][:, 512:1024],
                            start=(ci == 0), stop=(ci == NDT - 1))
                    yt = p2.tile([128, D], BF16, name="y_t", tag="y_t",
                                 bufs=3)
                    with nc.allow_low_precision(reason="bf16 y output"):
                        nc.vector.tensor_add(yt[:, 0:512], py[:], bo_sb[:, 0:512])
                        nc.vector.tensor_add(yt[:, 512:1024], py2[:],
                                             bo_sb[:, 512:1024])
                    nc.sync.dma_start(y[tt * 128:(tt + 1) * 128, :], yt[:])
                pending_norm = []
                pending_proj = []
                for ti in range(NDT):
                    h0d = (2 * ti) * (DK + 1)      # head col offsets in vv
                    h1d = h0d + (DK + 1)
                    for tb in range(NTB):
                        tbs = slice(tb * 512, (tb + 1) * 512)
                        pcA = p2pc.tile([65, 512], F32, name="pcA", tag="pc")
                        pcB = p2pc.tile([65, 512], F32, name="pcB", tag="pc")

                        def emit_scores_exp(si):
                            ss = slice(si * 128, (si + 1) * 128)
                            evs = []
                            for half in range(2):
                                sp = p2sp.tile([128, 512], F32, name="sp",
                                               tag="sp")
                                nc.tensor.matmul(
                                    sp[:],
                                    kT[ti][half * 64:half * 64 + 64, ss],
                                    qT[ti][half * 64:half * 64 + 64, tbs],
                                    start=True, stop=True)
                                if si in ACT_BOTH or (si + half) % 2 == 0:
                                    et = p2.tile([128, 512], BF16, name="etA",
                                                 tag="etA", bufs=8)
                                    nc.scalar.activation(et[:], sp[:],
                                                         Exp, scale=SCALE,
                                                         bias=actbias_sb[:])
                                    evs.append(et[:])
                                else:
                                    et = p2.tile([128, 512], I16, name="etS",
                                                 tag="etS", bufs=8)
                                    nc.vector.tensor_scalar(et[:], sp[:],
                                                            A16, B16S,
                                                            AT.mult, AT.add)
                                    evs.append(et[:].bitcast(BF16))
                            return evs

                        def emit_pv(si, evs):
                            nc.tensor.matmul(pcA[:],
                                             vv[si][:, h0d:h0d + DK + 1],
                                             evs[0],
                                             start=(si == 0),
                                             stop=(si == NSI - 1))
                            nc.tensor.matmul(pcB[:],
                                             vv[si][:, h1d:h1d + DK + 1],
                                             evs[1],
                                             start=(si == 0),
                                             stop=(si == NSI - 1))

                        pend = []
                        for si in range(NSI):
                            pend.append((si, emit_scores_exp(si)))
                            if len(pend) > 2:
                                emit_pv(*pend.pop(0))
                            if pending_norm and si >= 4 and pending_norm:
                                pending_norm.pop(0)()
                        for item in pend:
                            emit_pv(*item)
                        while pending_proj:
                            emit_proj(pending_proj.pop(0))
                        while pending_norm:
                            pending_norm.pop(0)()

                        # ---- normalization (spread into the next block) ----
                        def make_norm(ti, tbs, pcA, pcB):
                            st = {}

                            def n1():
                                st['dsum'] = pp.tile([1, 1024], F32,
                                                     name="dsum", tag="dsum",
                                                     bufs=2)
                                nc.vector.tensor_copy(st['dsum'][:, 0:512],
                                                      pcA[64:65, :])

                            def n2():
                                nc.vector.tensor_copy(st['dsum'][:, 512:1024],
                                                      pcB[64:65, :])

                            def n3():
                                st['rci'] = pp.tile([1, 1024], I32,
                                                    name="rci", tag="rci",
                                                    bufs=2)
                                nc.vector.tensor_scalar(
                                    st['rci'][:], st['dsum'][:].bitcast(I32),
                                    -1.0, C_RECIP, AT.mult, AT.add)

                            def n4():
                                st['aa'] = pp.tile([1, 1024], F32, name="ra",
                                                   tag="ra", bufs=2)
                                nc.vector.tensor_mul(
                                    st['aa'][:], st['dsum'][:],
                                    st['rci'][:].bitcast(F32))

                            def n5():
                                st['cc'] = pp.tile([1, 1024], F32, name="rb2",
                                                   tag="rb2", bufs=2)
                                nc.vector.tensor_mul(
                                    st['cc'][:], st['aa'][:],
                                    st['rci'][:].bitcast(F32))

                            def n6():
                                st['r1'] = pp.tile([1, 1024], F32, name="r1",
                                                   tag="r1", bufs=2)
                                nc.vector.scalar_tensor_tensor(
                                    st['r1'][:], st['rci'][:].bitcast(F32),
                                    NR_SCALAR, st['cc'][:], AT.mult,
                                    AT.subtract)
                                st['rb'] = pp.tile([64, 1024], F32,
                                                   name="rbb", tag="rbb",
                                                   bufs=2)
                                nc.gpsimd.partition_broadcast(st['rb'][:],
                                                              st['r1'][:])

                            def n7():
                                with nc.allow_low_precision(reason="bf16 ctx"):
                                    nc.vector.tensor_mul(ctx[ti][0:64, tbs],
                                                         pcA[0:64, :],
                                                         st['rb'][:, 0:512])

                            def n8():
                                with nc.allow_low_precision(reason="bf16 ctx"):
                                    nc.vector.tensor_mul(ctx[ti][64:128, tbs],
                                                         pcB[0:64, :],
                                                         st['rb'][:, 512:1024])

                            return [n1, n2, n3, n4, n5, n6, n7, n8]

                        pending_norm = make_norm(ti, tbs, pcA, pcB)
                        if ti == NDT - 1 and tb > 0:
                            for tt in range((tb - 1) * 4, tb * 4):
                                pending_proj.append(tt)
                while pending_proj:
                    emit_proj(pending_proj.pop(0))
                while pending_norm:
                    pending_norm.pop(0)()
                for tt in range(12, 16):
                    emit_proj(tt)

    nc.compile()
    return nc


_PROGRAM = None


def _get_program():
    global _PROGRAM
    if _PROGRAM is None:
        _PROGRAM = build_program()
    return _PROGRAM


def make_in_maps(x, w_qkv, b_qkv, w_out, b_out):
    x = np.asarray(x, dtype=np.float32)
    w_qkv = np.asarray(w_qkv, dtype=np.float32)
    b_qkv = np.asarray(b_qkv, dtype=np.float32)
    w_out = np.asarray(w_out, dtype=np.float32)
    b_out = np.asarray(b_out, dtype=np.float32)
    tobf = lambda a: np.ascontiguousarray(a).astype(ml_dtypes.bfloat16)

    in_maps = []
    for core in range(NCORES):
        b, g = divmod(core, G)
        gs = slice(g * HD, (g + 1) * HD)
        bo_part = b_out if g == 0 else np.zeros_like(b_out)
        in_maps.append({
            "xt": tobf(x[b].T),
            "wq": tobf(w_qkv[:, 0 * D:1 * D][:, gs]),
            "wk": tobf(w_qkv[:, 1 * D:2 * D][:, gs]),
            "wv": tobf(w_qkv[:, 2 * D:3 * D][:, gs]),
            "bq": np.ascontiguousarray(b_qkv[0 * D:1 * D][gs].reshape(HD, 1)),
            "bk": np.ascontiguousarray(b_qkv[1 * D:2 * D][gs].reshape(HD, 1)),
            "bv": np.ascontiguousarray(
                np.broadcast_to(b_qkv[2 * D:3 * D][gs], (128, HD)).astype(np.float32)),
            "wo": tobf(w_out[gs, :]),
            "bo": np.ascontiguousarray(
                np.broadcast_to(bo_part, (128, D)).astype(np.float32)),
        })
    return in_maps


def run(inputs, trace=False, tmpdir=None):
    nc = _get_program()
    in_maps = make_in_maps(**inputs)
    res = run_bass_kernel_spmd(nc, in_maps, list(range(NCORES)),
                               trace=trace, tmpdir=tmpdir)
    parts = [np.asarray(res.results[c]["y"]).astype(np.float32)
             for c in range(NCORES)]
    out = np.empty((B, T, D), dtype=np.float32)
    for b in range(B):
        out[b] = parts[b * G + 0] + parts[b * G + 1]
    return out, res


def kernel(**inputs) -> np.ndarray:
    out, _ = run(inputs, trace=False)
    return out
